# revision 8
# baseline (speedup 1.0000x reference)
"""GPT-Neo self-attention on 8 Trainium2 NeuronCores.

Sharding: (batch=2) x (head-groups=4) -> 8 cores; each core handles 4 of the
16 heads for one batch element. Out-projection is row-parallel: each core
produces a partial [S, E] output, summed on the host.

Device dataflow per core (all operands pre-transposed on host so the
contraction dim always lands on SBUF partitions):
  qT/kT[f, s]  = sum_e WT[e, f]^T @ xT[e, s]          (PSUM [f=128, s<=512])
  v[s, f]      = sum_e xT[e, s]^T  @ WvT[e, f]         (PSUM [s=128, f=256])
  sT[j, i]     = kT_h[d, j]^T @ qT_h[d, i]             (PSUM [j=128, i=512])
  p = exp(sT + causal_mask)                            (ACT; no max-subtract:
                                                        |scores| < ~60 so fp32
                                                        exp cannot overflow)
  attnT[d, i] += v_h[j, d]^T @ p[j, i]                 (PSUM accum over j)
  denom[1, i] += ones[j, 1]^T @ p[j, i]
  attnT *= 1/denom ; out[i, f] += attnT[:, h, i]^T @ WoT[e2, f]

Fully-masked causal blocks are skipped (half the attention FLOPs).

Precision configs (softmax amplifies absolute score error, so the
q/k-projection + scores path needs more precision than the linear P@V path):
  S: everything fp32 (4 cyc/row on PE - slow, exact)
  B: sensitive path bf16 hi/lo split (3 matmuls, ~2^-18 effective rounding),
     linear path bf16
  C: everything fp32r (1 cyc/row at N>=256, tf32-class rounding)
"""

import os
import sys

sys.path.insert(0, "/opt/trn_rl_repo")

import numpy as np
import ml_dtypes

import concourse.bass as bass
import concourse.mybir as mybir
import concourse.tile as tile
from concourse import bacc
from concourse.bass_utils import run_bass_kernel_spmd

B, S, E, H, D = 2, 2048, 2048, 16, 128
P = 128
NCORES = 8
NH = H * B // NCORES      # heads per core = 4
FH = NH * D               # per-core projected width = 512
SC = 512                  # i-chunk width (max fp32 moving dim / psum bank)
EO = E // P               # 16 contraction chunks
MASK_NEG = -1e30

F32 = mybir.dt.float32
BF16 = mybir.dt.bfloat16
F32R = mybir.dt.float32r
EXPF = mybir.ActivationFunctionType.Exp

CFG = os.environ.get("ATTN_CFG", "S")


def _dma_in(nc, dst, src_ap, dt_sens):
    """DMA a fp32 DRAM AP into an SBUF tile, bitcasting for fp32r tiles."""
    if dst.dtype == F32R and src_ap.dtype == F32:
        src_ap = src_ap.bitcast(F32R)
    nc.sync.dma_start(dst, src_ap)


def _build(cfg):
    nc = bacc.Bacc("TRN2", target_bir_lowering=False)

    if cfg == "B":
        dt_sens, dt_lin, n_rep = BF16, BF16, 2
    elif cfg == "C":
        dt_sens, dt_lin, n_rep = F32R, F32R, 1
    else:
        dt_sens, dt_lin, n_rep = F32, F32, 1
    dt_in = BF16 if cfg == "B" else F32   # dtype arriving from host
    proj_terms = [(0, 0), (0, 1), (1, 0)] if n_rep == 2 else [(0, 0)]

    reps = range(n_rep)
    xts = [nc.dram_tensor(f"xt{r}", [E, S], dt_in, kind="ExternalInput") for r in reps]
    wqs = [nc.dram_tensor(f"wq{r}", [E, FH], dt_in, kind="ExternalInput") for r in reps]
    wks = [nc.dram_tensor(f"wk{r}", [E, FH], dt_in, kind="ExternalInput") for r in reps]
    wv = nc.dram_tensor("wv", [E, FH], dt_in, kind="ExternalInput")
    wo = nc.dram_tensor("wo", [FH, E], dt_in, kind="ExternalInput")
    out = nc.dram_tensor("out", [S, E], F32, kind="ExternalOutput")
    # qT/kT spill (per-core DRAM scratch)
    qts = [nc.dram_tensor(f"qts{r}", [FH, S], dt_sens) for r in reps]
    kts = [nc.dram_tensor(f"kts{r}", [FH, S], dt_sens) for r in reps]

    with tile.TileContext(nc) as tc:
        with tc.tile_pool(name="pers", bufs=1) as pers:
            # ---- persistent tiles: v, causal masks, ones ----
            vsb = pers.tile([P, S // P, FH], dt_lin, tag="v")
            ones = pers.tile([P, P], dt_lin, tag="ones")
            nc.vector.memset(ones[:], 1.0)
            # constant bias for exp(S - 30): keeps exp in fp32 range without
            # changing softmax ratios (max |S| here is ~90 > ln(fp32max))
            ebias = pers.tile([P, 1], F32, tag="ebias")
            nc.vector.memset(ebias[:], -30.0)
            # one wide causal mask; mask for diag-offset m is the slice
            # [:, 384-128m : 896-128m]:  keep where y >= p + 384
            mwide = pers.tile([P, SC + 384], F32, tag="mw")
            nc.gpsimd.memset(mwide[:], 0.0)
            nc.gpsimd.affine_select(
                out=mwide[:], in_=mwide[:], compare_op=mybir.AluOpType.is_ge,
                fill=MASK_NEG, base=-384, pattern=[[1, SC + 384]],
                channel_multiplier=-1)
            masks = [mwide[:, 384 - P * m:384 - P * m + SC] for m in range(4)]

            # ---------------- Phase A: q/k/v projections ----------------
            # x processed in s-halves to fit SBUF (weights loaded twice)
            SH = S // 2
            with tc.tile_pool(name="xp", bufs=1) as xp, \
                 tc.tile_pool(name="wp", bufs=2) as wp, \
                 tc.tile_pool(name="wvp", bufs=1) as wvp, \
                 tc.tile_pool(name="spl", bufs=3) as spl, \
                 tc.tile_pool(name="pa", bufs=2, space="PSUM") as pa:

                for sh in range(2):
                    s0 = sh * SH
                    xsb = []
                    for r in reps:
                        t = xp.tile([P, EO, SH], dt_sens, tag=f"x{r}")
                        _dma_in(nc, t[:],
                                xts[r].rearrange("(eo ei) s -> ei eo s", ei=P)[
                                    :, :, s0:s0 + SH],
                                dt_sens)
                        xsb.append(t)

                    # q/k projections, streamed weight column-tiles
                    for wdrams, spill in ((wqs, qts), (wks, kts)):
                        for fc in range(FH // P):        # 4 chunks of 128
                            wtiles = []
                            for r in reps:
                                wt = wp.tile([P, EO, P], dt_sens, tag=f"w{r}")
                                _dma_in(
                                    nc, wt[:],
                                    wdrams[r].rearrange(
                                        "(eo ei) f -> ei eo f", ei=P)[
                                        :, :, fc * P:(fc + 1) * P],
                                    dt_sens)
                                wtiles.append(wt)
                            for sc_ in range(SH // SC):   # 2 chunks of 512
                                pt = pa.tile([P, SC], F32, tag="pqk")
                                n_mm = EO * len(proj_terms)
                                i_mm = 0
                                for e in range(EO):
                                    for (rx, rw) in proj_terms:
                                        nc.tensor.matmul(
                                            pt[:],
                                            wtiles[rw][:, e, :],
                                            xsb[rx][:, e,
                                                    sc_ * SC:(sc_ + 1) * SC],
                                            start=(i_mm == 0),
                                            stop=(i_mm == n_mm - 1))
                                        i_mm += 1
                                # split / cast and spill to DRAM
                                cols = slice(s0 + sc_ * SC, s0 + (sc_ + 1) * SC)
                                hi = spl.tile([P, SC], dt_sens, tag="hi")
                                nc.vector.tensor_copy(out=hi[:], in_=pt[:])
                                nc.sync.dma_start(
                                    spill[0][fc * P:(fc + 1) * P, cols], hi[:])
                                if n_rep == 2:
                                    lo = spl.tile([P, SC], dt_sens, tag="lo")
                                    nc.vector.tensor_tensor(
                                        out=lo[:], in0=pt[:], in1=hi[:],
                                        op=mybir.AluOpType.subtract)
                                    nc.sync.dma_start(
                                        spill[1][fc * P:(fc + 1) * P, cols],
                                        lo[:])

                    # v projection in two half-width passes (wv half resident)
                    for fq in range(2):
                        FQ = FH // 2  # 256
                        wvsb = wvp.tile([P, EO, FQ], dt_lin, tag="wv")
                        _dma_in(nc, wvsb[:],
                                wv.rearrange("(eo ei) f -> ei eo f", ei=P)[
                                    :, :, fq * FQ:(fq + 1) * FQ],
                                dt_lin)
                        for sc128 in range(SH // P):
                            pt = pa.tile([P, FQ], F32, tag="pv")
                            for e in range(EO):
                                nc.tensor.matmul(
                                    pt[:],
                                    xsb[0][:, e, sc128 * P:(sc128 + 1) * P],
                                    wvsb[:, e, :],
                                    start=(e == 0), stop=(e == EO - 1))
                            nc.vector.tensor_copy(
                                out=vsb[:, sh * (SH // P) + sc128,
                                        fq * FQ:(fq + 1) * FQ],
                                in_=pt[:])

            # ---------------- Phase C: attention ----------------
            score_terms = proj_terms
            with tc.tile_pool(name="att", bufs=1) as att:
                attnsb = att.tile([P, NH, S], dt_lin, tag="attn")
                with tc.tile_pool(name="qk", bufs=2) as qk, \
                     tc.tile_pool(name="esp", bufs=4) as esp, \
                     tc.tile_pool(name="tmpp", bufs=3) as tmpp, \
                     tc.tile_pool(name="psc", bufs=3, space="PSUM") as psc, \
                     tc.tile_pool(name="pau", bufs=2, space="PSUM") as pau, \
                     tc.tile_pool(name="pdn", bufs=2, space="PSUM") as pdn:

                    for h in range(NH):
                        qreps, kreps = [], []
                        for r in reps:
                            qt = qk.tile([P, S], dt_sens, tag=f"qh{r}")
                            nc.sync.dma_start(qt[:], qts[r][h * P:(h + 1) * P, :])
                            qreps.append(qt)
                            kt = qk.tile([P, S], dt_sens, tag=f"kh{r}")
                            nc.sync.dma_start(kt[:], kts[r][h * P:(h + 1) * P, :])
                            kreps.append(kt)
                        for ic in range(S // SC):
                            njc = 4 * ic + 4
                            au = pau.tile([P, SC], F32, tag="au")
                            dn = pdn.tile([P, SC], F32, tag="dn")
                            es_tiles = [None] * njc

                            def emit_scores(jc, ic=ic, qreps=qreps, kreps=kreps,
                                            es_tiles=es_tiles):
                                sp = psc.tile([P, SC], F32, tag="sp")
                                for t, (rk, rq) in enumerate(score_terms):
                                    nc.tensor.matmul(
                                        sp[:],
                                        kreps[rk][:, jc * P:(jc + 1) * P],
                                        qreps[rq][:, ic * SC:(ic + 1) * SC],
                                        start=(t == 0),
                                        stop=(t == len(score_terms) - 1))
                                es = esp.tile([P, SC], dt_lin, tag="es")
                                doff = jc - 4 * ic
                                if doff >= 0:
                                    tmp = tmpp.tile([P, SC], F32, tag="tmp")
                                    nc.vector.tensor_tensor(
                                        out=tmp[:], in0=sp[:], in1=masks[doff][:],
                                        op=mybir.AluOpType.add)
                                    nc.scalar.activation(
                                        out=es[:], in_=tmp[:], func=EXPF,
                                        bias=ebias[:])
                                else:
                                    nc.scalar.activation(
                                        out=es[:], in_=sp[:], func=EXPF,
                                        bias=ebias[:])
                                es_tiles[jc] = es

                            def emit_accum(jc, h=h, au=au, dn=dn,
                                           es_tiles=es_tiles, njc=njc):
                                es = es_tiles[jc]
                                nc.tensor.matmul(
                                    au[:], vsb[:, jc, h * D:(h + 1) * D], es[:],
                                    start=(jc == 0), stop=(jc == njc - 1))
                                nc.tensor.matmul(
                                    dn[:], ones[:], es[:],
                                    start=(jc == 0), stop=(jc == njc - 1))

                            # software pipeline: scores(jc+1) before accum(jc)
                            emit_scores(0)
                            for jc in range(1, njc):
                                emit_scores(jc)
                                emit_accum(jc - 1)
                            emit_accum(njc - 1)

                            rcb = tmpp.tile([P, SC], F32, tag="rcb")
                            nc.vector.reciprocal(rcb[:], dn[:])
                            nc.vector.tensor_tensor(
                                out=attnsb[:, h, ic * SC:(ic + 1) * SC],
                                in0=au[:], in1=rcb[:],
                                op=mybir.AluOpType.mult)

                # ---------------- Phase D: out projection ----------------
                with tc.tile_pool(name="wop", bufs=1) as wop, \
                     tc.tile_pool(name="op", bufs=4) as op, \
                     tc.tile_pool(name="pd", bufs=4, space="PSUM") as pd:
                    wosb = wop.tile([P, NH, E], dt_lin, tag="wo")
                    _dma_in(nc, wosb[:],
                            wo.rearrange("(ho hi) f -> hi ho f", hi=P),
                            dt_lin)
                    for ic128 in range(S // P):
                        for fc in range(E // SC):
                            po = pd.tile([P, SC], F32, tag="po")
                            for h in range(NH):
                                nc.tensor.matmul(
                                    po[:],
                                    attnsb[:, h, ic128 * P:(ic128 + 1) * P],
                                    wosb[:, h, fc * SC:(fc + 1) * SC],
                                    start=(h == 0), stop=(h == NH - 1))
                            ot = op.tile([P, SC], F32, tag="ot")
                            nc.scalar.activation(
                                out=ot[:], in_=po[:],
                                func=mybir.ActivationFunctionType.Copy)
                            nc.sync.dma_start(
                                out[ic128 * P:(ic128 + 1) * P,
                                    fc * SC:(fc + 1) * SC],
                                ot[:])

    nc.compile()
    return nc


_prog_cache = {}


def _get_prog(cfg):
    if cfg not in _prog_cache:
        _prog_cache[cfg] = _build(cfg)
    return _prog_cache[cfg]


def _split_bf16(a):
    hi = a.astype(ml_dtypes.bfloat16)
    lo = (a - hi.astype(np.float32)).astype(ml_dtypes.bfloat16)
    return hi, lo


def kernel(hidden_states, Wq, Wk, Wv, Wo, bo):
    cfg = CFG
    nc = _get_prog(cfg)

    hidden_states = np.asarray(hidden_states, dtype=np.float32)
    xt_b = [np.ascontiguousarray(hidden_states[b].T) for b in range(B)]
    if cfg == "B":
        xt_split = [_split_bf16(x) for x in xt_b]

    in_maps = []
    for c in range(NCORES):
        b, g = c // (NCORES // B), c % (NCORES // B)
        sl = slice(g * FH, (g + 1) * FH)
        wq_t = np.ascontiguousarray(np.asarray(Wq)[sl, :].T)   # [E, FH]
        wk_t = np.ascontiguousarray(np.asarray(Wk)[sl, :].T)
        wv_t = np.ascontiguousarray(np.asarray(Wv)[sl, :].T)
        wo_t = np.ascontiguousarray(np.asarray(Wo)[:, sl].T)   # [FH, E]
        if cfg == "B":
            xh, xl = xt_split[b]
            qh, ql = _split_bf16(wq_t)
            kh, kl = _split_bf16(wk_t)
            m = {"xt0": xh, "xt1": xl, "wq0": qh, "wq1": ql,
                 "wk0": kh, "wk1": kl,
                 "wv": wv_t.astype(ml_dtypes.bfloat16),
                 "wo": wo_t.astype(ml_dtypes.bfloat16)}
        else:
            m = {"xt0": xt_b[b], "wq0": wq_t, "wk0": wk_t,
                 "wv": wv_t, "wo": wo_t}
        in_maps.append(m)

    res = run_bass_kernel_spmd(nc, in_maps, list(range(NCORES)))
    out = np.zeros((B, S, E), dtype=np.float32)
    for c in range(NCORES):
        out[c // (NCORES // B)] += res.results[c]["out"]
    out += np.asarray(bo, dtype=np.float32)[None, None, :]
    return out


# revision 9
# speedup vs baseline: 1.4498x; 1.4498x over previous
"""GPT-Neo self-attention on 8 Trainium2 NeuronCores.

Sharding: (batch=2) x (head-groups=4) -> 8 cores; each core handles 4 of the
16 heads for one batch element. Out-projection is row-parallel: each core
produces a partial [S, E] output, summed on the host.

Device dataflow per core (all operands pre-transposed on host so the
contraction dim always lands on SBUF partitions):
  qT/kT[f, s]  = sum_e WT[e, f]^T @ xT[e, s]          (PSUM [f=128, s<=512])
  v[s, f]      = sum_e xT[e, s]^T  @ WvT[e, f]         (PSUM [s=128, f=256])
  sT[j, i]     = kT_h[d, j]^T @ qT_h[d, i]             (PSUM [j=128, i=512])
  p = exp(sT + causal_mask)                            (ACT; no max-subtract:
                                                        |scores| < ~60 so fp32
                                                        exp cannot overflow)
  attnT[d, i] += v_h[j, d]^T @ p[j, i]                 (PSUM accum over j)
  denom[1, i] += ones[j, 1]^T @ p[j, i]
  attnT *= 1/denom ; out[i, f] += attnT[:, h, i]^T @ WoT[e2, f]

Fully-masked causal blocks are skipped (half the attention FLOPs).

Precision configs (softmax amplifies absolute score error, so the
q/k-projection + scores path needs more precision than the linear P@V path):
  S: everything fp32 (4 cyc/row on PE - slow, exact)
  B: sensitive path bf16 hi/lo split (3 matmuls, ~2^-18 effective rounding),
     linear path bf16
  C: everything fp32r (1 cyc/row at N>=256, tf32-class rounding)
"""

import os
import sys

sys.path.insert(0, "/opt/trn_rl_repo")

import numpy as np
import ml_dtypes

import concourse.bass as bass
import concourse.mybir as mybir
import concourse.tile as tile
from concourse import bacc
from concourse.bass_utils import run_bass_kernel_spmd

B, S, E, H, D = 2, 2048, 2048, 16, 128
P = 128
NCORES = 8
NH = H * B // NCORES      # heads per core = 4
FH = NH * D               # per-core projected width = 512
SC = 512                  # i-chunk width (max fp32 moving dim / psum bank)
EO = E // P               # 16 contraction chunks
MASK_NEG = -1e30

F32 = mybir.dt.float32
BF16 = mybir.dt.bfloat16
F32R = mybir.dt.float32r
EXPF = mybir.ActivationFunctionType.Exp

CFG = os.environ.get("ATTN_CFG", "S")


def _dma_in(nc, dst, src_ap, dt_sens):
    """DMA a fp32 DRAM AP into an SBUF tile, bitcasting for fp32r tiles."""
    if dst.dtype == F32R and src_ap.dtype == F32:
        src_ap = src_ap.bitcast(F32R)
    nc.sync.dma_start(dst, src_ap)


def _build(cfg):
    nc = bacc.Bacc("TRN2", target_bir_lowering=False)

    if cfg == "B":
        dt_sens, dt_lin, n_rep = BF16, BF16, 2
    elif cfg == "C":
        dt_sens, dt_lin, n_rep = F32R, F32R, 1
    else:
        dt_sens, dt_lin, n_rep = F32, F32, 1
    dt_in = BF16 if cfg == "B" else F32   # dtype arriving from host
    proj_terms = [(0, 0), (0, 1), (1, 0)] if n_rep == 2 else [(0, 0)]

    reps = range(n_rep)
    xts = [nc.dram_tensor(f"xt{r}", [E, S], dt_in, kind="ExternalInput") for r in reps]
    wqs = [nc.dram_tensor(f"wq{r}", [E, FH], dt_in, kind="ExternalInput") for r in reps]
    wks = [nc.dram_tensor(f"wk{r}", [E, FH], dt_in, kind="ExternalInput") for r in reps]
    wv = nc.dram_tensor("wv", [E, FH], dt_in, kind="ExternalInput")
    wo = nc.dram_tensor("wo", [FH, E], dt_in, kind="ExternalInput")
    out = nc.dram_tensor("out", [S, E], F32, kind="ExternalOutput")
    # qT/kT spill (per-core DRAM scratch)
    qts = [nc.dram_tensor(f"qts{r}", [FH, S], dt_sens) for r in reps]
    kts = [nc.dram_tensor(f"kts{r}", [FH, S], dt_sens) for r in reps]

    with tile.TileContext(nc) as tc:
        with tc.tile_pool(name="pers", bufs=1) as pers:
            # ---- persistent tiles: v, causal masks, ones ----
            vsb = pers.tile([P, S // P, FH], dt_lin, tag="v")
            ones = pers.tile([P, P], dt_lin, tag="ones")
            if dt_lin == F32R:
                ones_f = pers.tile([P, P], F32, tag="ones_f")
                nc.vector.memset(ones_f[:], 1.0)
                nc.vector.tensor_copy(out=ones[:], in_=ones_f[:])
            else:
                nc.vector.memset(ones[:], 1.0)
            # constant bias for exp(S - 30): keeps exp in fp32 range without
            # changing softmax ratios (max |S| here is ~90 > ln(fp32max))
            ebias = pers.tile([P, 1], F32, tag="ebias")
            nc.vector.memset(ebias[:], -30.0)
            # one wide causal mask; mask for diag-offset m is the slice
            # [:, 384-128m : 896-128m]:  keep where y >= p + 384
            mwide = pers.tile([P, SC + 384], F32, tag="mw")
            nc.gpsimd.memset(mwide[:], 0.0)
            nc.gpsimd.affine_select(
                out=mwide[:], in_=mwide[:], compare_op=mybir.AluOpType.is_ge,
                fill=MASK_NEG, base=-384, pattern=[[1, SC + 384]],
                channel_multiplier=-1)
            masks = [mwide[:, 384 - P * m:384 - P * m + SC] for m in range(4)]

            # ---------------- Phase A: q/k/v projections ----------------
            # x processed in s-halves to fit SBUF (weights loaded twice)
            SH = S // 2
            with tc.tile_pool(name="xp", bufs=1) as xp, \
                 tc.tile_pool(name="wp", bufs=2) as wp, \
                 tc.tile_pool(name="wvp", bufs=1) as wvp, \
                 tc.tile_pool(name="spl", bufs=3) as spl, \
                 tc.tile_pool(name="pa", bufs=2, space="PSUM") as pa:

                for sh in range(2):
                    s0 = sh * SH
                    xsb = []
                    for r in reps:
                        t = xp.tile([P, EO, SH], dt_sens, tag=f"x{r}")
                        _dma_in(nc, t[:],
                                xts[r].rearrange("(eo ei) s -> ei eo s", ei=P)[
                                    :, :, s0:s0 + SH],
                                dt_sens)
                        xsb.append(t)

                    # q/k projections, streamed weight column-tiles
                    for wdrams, spill in ((wqs, qts), (wks, kts)):
                        for fc in range(FH // P):        # 4 chunks of 128
                            wtiles = []
                            for r in reps:
                                wt = wp.tile([P, EO, P], dt_sens, tag=f"w{r}")
                                _dma_in(
                                    nc, wt[:],
                                    wdrams[r].rearrange(
                                        "(eo ei) f -> ei eo f", ei=P)[
                                        :, :, fc * P:(fc + 1) * P],
                                    dt_sens)
                                wtiles.append(wt)
                            for sc_ in range(SH // SC):   # 2 chunks of 512
                                pt = pa.tile([P, SC], F32, tag="pqk")
                                n_mm = EO * len(proj_terms)
                                i_mm = 0
                                for e in range(EO):
                                    for (rx, rw) in proj_terms:
                                        nc.tensor.matmul(
                                            pt[:],
                                            wtiles[rw][:, e, :],
                                            xsb[rx][:, e,
                                                    sc_ * SC:(sc_ + 1) * SC],
                                            start=(i_mm == 0),
                                            stop=(i_mm == n_mm - 1))
                                        i_mm += 1
                                # split / cast and spill to DRAM
                                cols = slice(s0 + sc_ * SC, s0 + (sc_ + 1) * SC)
                                hi = spl.tile([P, SC], dt_sens, tag="hi")
                                nc.vector.tensor_copy(out=hi[:], in_=pt[:])
                                nc.sync.dma_start(
                                    spill[0][fc * P:(fc + 1) * P, cols], hi[:])
                                if n_rep == 2:
                                    lo = spl.tile([P, SC], dt_sens, tag="lo")
                                    nc.vector.tensor_tensor(
                                        out=lo[:], in0=pt[:], in1=hi[:],
                                        op=mybir.AluOpType.subtract)
                                    nc.sync.dma_start(
                                        spill[1][fc * P:(fc + 1) * P, cols],
                                        lo[:])

                    # v projection in two half-width passes (wv half resident)
                    for fq in range(2):
                        FQ = FH // 2  # 256
                        wvsb = wvp.tile([P, EO, FQ], dt_lin, tag="wv")
                        _dma_in(nc, wvsb[:],
                                wv.rearrange("(eo ei) f -> ei eo f", ei=P)[
                                    :, :, fq * FQ:(fq + 1) * FQ],
                                dt_lin)
                        for sc128 in range(SH // P):
                            pt = pa.tile([P, FQ], F32, tag="pv")
                            for e in range(EO):
                                nc.tensor.matmul(
                                    pt[:],
                                    xsb[0][:, e, sc128 * P:(sc128 + 1) * P],
                                    wvsb[:, e, :],
                                    start=(e == 0), stop=(e == EO - 1))
                            nc.vector.tensor_copy(
                                out=vsb[:, sh * (SH // P) + sc128,
                                        fq * FQ:(fq + 1) * FQ],
                                in_=pt[:])

            # ---------------- Phase C: attention ----------------
            score_terms = proj_terms
            with tc.tile_pool(name="att", bufs=1) as att:
                attnsb = att.tile([P, NH, S], dt_lin, tag="attn")
                with tc.tile_pool(name="qk", bufs=2) as qk, \
                     tc.tile_pool(name="esp", bufs=4) as esp, \
                     tc.tile_pool(name="tmpp", bufs=3) as tmpp, \
                     tc.tile_pool(name="psc", bufs=3, space="PSUM") as psc, \
                     tc.tile_pool(name="pau", bufs=2, space="PSUM") as pau, \
                     tc.tile_pool(name="pdn", bufs=2, space="PSUM") as pdn:

                    for h in range(NH):
                        qreps, kreps = [], []
                        for r in reps:
                            qt = qk.tile([P, S], dt_sens, tag=f"qh{r}")
                            nc.sync.dma_start(qt[:], qts[r][h * P:(h + 1) * P, :])
                            qreps.append(qt)
                            kt = qk.tile([P, S], dt_sens, tag=f"kh{r}")
                            nc.sync.dma_start(kt[:], kts[r][h * P:(h + 1) * P, :])
                            kreps.append(kt)
                        for ic in range(S // SC):
                            njc = 4 * ic + 4
                            au = pau.tile([P, SC], F32, tag="au")
                            dn = pdn.tile([P, SC], F32, tag="dn")
                            es_tiles = [None] * njc

                            def emit_scores(jc, ic=ic, qreps=qreps, kreps=kreps,
                                            es_tiles=es_tiles):
                                sp = psc.tile([P, SC], F32, tag="sp")
                                for t, (rk, rq) in enumerate(score_terms):
                                    nc.tensor.matmul(
                                        sp[:],
                                        kreps[rk][:, jc * P:(jc + 1) * P],
                                        qreps[rq][:, ic * SC:(ic + 1) * SC],
                                        start=(t == 0),
                                        stop=(t == len(score_terms) - 1))
                                es = esp.tile([P, SC], dt_lin, tag="es")
                                doff = jc - 4 * ic
                                if doff >= 0:
                                    tmp = tmpp.tile([P, SC], F32, tag="tmp")
                                    nc.vector.tensor_tensor(
                                        out=tmp[:], in0=sp[:], in1=masks[doff][:],
                                        op=mybir.AluOpType.add)
                                    nc.scalar.activation(
                                        out=es[:], in_=tmp[:], func=EXPF,
                                        bias=ebias[:])
                                else:
                                    nc.scalar.activation(
                                        out=es[:], in_=sp[:], func=EXPF,
                                        bias=ebias[:])
                                es_tiles[jc] = es

                            def emit_accum(jc, h=h, au=au, dn=dn,
                                           es_tiles=es_tiles, njc=njc):
                                es = es_tiles[jc]
                                nc.tensor.matmul(
                                    au[:], vsb[:, jc, h * D:(h + 1) * D], es[:],
                                    start=(jc == 0), stop=(jc == njc - 1))
                                nc.tensor.matmul(
                                    dn[:], ones[:], es[:],
                                    start=(jc == 0), stop=(jc == njc - 1))

                            # software pipeline: scores(jc+1) before accum(jc)
                            emit_scores(0)
                            for jc in range(1, njc):
                                emit_scores(jc)
                                emit_accum(jc - 1)
                            emit_accum(njc - 1)

                            rcb = tmpp.tile([P, SC], F32, tag="rcb")
                            nc.vector.reciprocal(rcb[:], dn[:])
                            nc.vector.tensor_tensor(
                                out=attnsb[:, h, ic * SC:(ic + 1) * SC],
                                in0=au[:], in1=rcb[:],
                                op=mybir.AluOpType.mult)

                # ---------------- Phase D: out projection ----------------
                with tc.tile_pool(name="wop", bufs=1) as wop, \
                     tc.tile_pool(name="op", bufs=4) as op, \
                     tc.tile_pool(name="pd", bufs=4, space="PSUM") as pd:
                    wosb = wop.tile([P, NH, E], dt_lin, tag="wo")
                    _dma_in(nc, wosb[:],
                            wo.rearrange("(ho hi) f -> hi ho f", hi=P),
                            dt_lin)
                    for ic128 in range(S // P):
                        for fc in range(E // SC):
                            po = pd.tile([P, SC], F32, tag="po")
                            for h in range(NH):
                                nc.tensor.matmul(
                                    po[:],
                                    attnsb[:, h, ic128 * P:(ic128 + 1) * P],
                                    wosb[:, h, fc * SC:(fc + 1) * SC],
                                    start=(h == 0), stop=(h == NH - 1))
                            ot = op.tile([P, SC], F32, tag="ot")
                            nc.scalar.activation(
                                out=ot[:], in_=po[:],
                                func=mybir.ActivationFunctionType.Copy)
                            nc.sync.dma_start(
                                out[ic128 * P:(ic128 + 1) * P,
                                    fc * SC:(fc + 1) * SC],
                                ot[:])

    nc.compile()
    return nc


_prog_cache = {}


def _get_prog(cfg):
    if cfg not in _prog_cache:
        _prog_cache[cfg] = _build(cfg)
    return _prog_cache[cfg]


def _split_bf16(a):
    hi = a.astype(ml_dtypes.bfloat16)
    lo = (a - hi.astype(np.float32)).astype(ml_dtypes.bfloat16)
    return hi, lo


def kernel(hidden_states, Wq, Wk, Wv, Wo, bo):
    cfg = CFG
    nc = _get_prog(cfg)

    hidden_states = np.asarray(hidden_states, dtype=np.float32)
    xt_b = [np.ascontiguousarray(hidden_states[b].T) for b in range(B)]
    if cfg == "B":
        xt_split = [_split_bf16(x) for x in xt_b]

    in_maps = []
    for c in range(NCORES):
        b, g = c // (NCORES // B), c % (NCORES // B)
        sl = slice(g * FH, (g + 1) * FH)
        wq_t = np.ascontiguousarray(np.asarray(Wq)[sl, :].T)   # [E, FH]
        wk_t = np.ascontiguousarray(np.asarray(Wk)[sl, :].T)
        wv_t = np.ascontiguousarray(np.asarray(Wv)[sl, :].T)
        wo_t = np.ascontiguousarray(np.asarray(Wo)[:, sl].T)   # [FH, E]
        if cfg == "B":
            xh, xl = xt_split[b]
            qh, ql = _split_bf16(wq_t)
            kh, kl = _split_bf16(wk_t)
            m = {"xt0": xh, "xt1": xl, "wq0": qh, "wq1": ql,
                 "wk0": kh, "wk1": kl,
                 "wv": wv_t.astype(ml_dtypes.bfloat16),
                 "wo": wo_t.astype(ml_dtypes.bfloat16)}
        else:
            m = {"xt0": xt_b[b], "wq0": wq_t, "wk0": wk_t,
                 "wv": wv_t, "wo": wo_t}
        in_maps.append(m)

    res = run_bass_kernel_spmd(nc, in_maps, list(range(NCORES)))
    out = np.zeros((B, S, E), dtype=np.float32)
    for c in range(NCORES):
        out[c // (NCORES // B)] += res.results[c]["out"]
    out += np.asarray(bo, dtype=np.float32)[None, None, :]
    return out
